# revision 13
# baseline (speedup 1.0000x reference)
"""GRU decoder (nn_Decoder) on 8 TRN2 NeuronCores — v2.

Layout / strategy:
- Host: fed tokens x_t known upfront (x_0=SOS, x_t=target[t-1]); gather
  embeddings [2048,512] on host, transpose+cast weights. All heavy matmuls run
  fp8e4 (TRN E4M3, max +-240) DoubleRow with power-of-2 scales S_H=32 (hidden)
  and S_W=1024 (weights); PSUM holds 32768*x, descaled for free by the
  activation input scale. w_ih prescaled by 32768 (bf16); n-gate rows of w_hh
  prescaled by 0.5 to fold the sigmoid->tanh rewrite.
- Gates use tanh only (sigmoid(x)=0.5*tanh(x/2)+0.5) so tanh AND exp live in
  the single `exp_and_others` activation table: zero table reloads.
- Phase A: h sharded 8x128 cols; per step, gi rows land in PSUM via an
  identity-extract matmul and the 8 k-chunk gh matmuls (fp8 DoubleRow, 4
  instrs) accumulate on top. Per-step AllGather of the fp8 h^T slice [128,36]
  (4 spare cols carry a bitcast f32 softmax partial sum piggybacked so only
  one standalone AllReduce is needed, at the tail). HWDGE (sync) staging.
- Phase B: vocab-sharded projection, fp8 DoubleRow [128 rows x 500-vocab
  tiles], exp on ACT with accum_out producing row sums, normalization sliced
  across steps on DVE with done-masking folded in, bf16 output. Vocab tiles
  interleave between recurrence steps (<=3 per step) so the PE stays warm.
"""

import numpy as np
import ml_dtypes

VOCAB = 32000
EMB = 512
HID = 1024
B = 32
SEQ = 64
PAD_ID = 0
SOS_ID = 1
EOS_ID = 2
NC = 8
HSL = HID // NC          # 128 h columns per core
GS = 3 * HSL             # 384 gate rows per core
RZ = 2 * HSL             # 256 r,z gate rows
VS = VOCAB // NC         # 4000 vocab per core
ROWS = SEQ * B           # 2048
MCH = ROWS // 128        # 16 m-chunks (4 steps each)
VT = 8                   # vocab tiles per m-chunk (4000 = 8*500)
VTW = VS // VT           # 500
NSL = 4                  # norm slices per chunk
NSW = VS // NSL          # 1000
AGW = B + 4              # AG payload cols: 32 h + 4 (bitcast f32 sum)
S_H = 32.0
S_W = 1024.0
S = S_H * S_W            # 32768

BF16 = ml_dtypes.bfloat16
FP8 = ml_dtypes.float8_e4m3

_CACHE = {}


def _build():
    import concourse.bass as bass
    import concourse.mybir as mybir
    import concourse.tile as tile
    from concourse import bacc
    from concourse.masks import make_identity

    f32 = mybir.dt.float32
    bf16 = mybir.dt.bfloat16
    fp8 = mybir.dt.float8e4
    AF = mybir.ActivationFunctionType
    ALU = mybir.AluOpType
    AX = mybir.AxisListType
    DR = mybir.MatmulPerfMode.DoubleRow

    nc = bacc.Bacc(None, target_bir_lowering=False, num_devices=NC)

    # ---- kernel I/O (per-core shards prepared by host) ----
    eT_d = nc.dram_tensor("eT", [EMB, ROWS], bf16, kind="ExternalInput")
    wihT_d = nc.dram_tensor("wihT", [EMB, GS], bf16, kind="ExternalInput")
    whh8_d = nc.dram_tensor("whh8", [HID, GS], fp8, kind="ExternalInput")
    wout8_d = nc.dram_tensor("wout8", [HID, VS], fp8, kind="ExternalInput")
    h8T0_d = nc.dram_tensor("h8T0", [HID, B], fp8, kind="ExternalInput")
    h0own_d = nc.dram_tensor("h0own", [B, HSL], f32, kind="ExternalInput")
    doneT_d = nc.dram_tensor("doneT", [B, SEQ], f32, kind="ExternalInput")
    live_d = nc.dram_tensor("livePB", [128, MCH], f32, kind="ExternalInput")
    pad_d = nc.dram_tensor("padPB", [128, MCH], f32, kind="ExternalInput")
    out_d = nc.dram_tensor("out", [ROWS, VS], bf16, kind="ExternalOutput")

    with tile.TileContext(nc) as tc:
        with (
            tc.tile_pool(name="wts", bufs=1) as wts,
            tc.tile_pool(name="state", bufs=1) as state,
            tc.tile_pool(name="hown", bufs=2) as hown_pool,
            tc.tile_pool(name="gtmp", bufs=4) as gtmp,
            tc.tile_pool(name="expp", bufs=3) as expp,
            tc.tile_pool(name="outp", bufs=2) as outp,
            tc.tile_pool(name="sums8", bufs=2) as sums8_pool,
            tc.tile_pool(name="pbp", bufs=2) as pbpool,
            tc.tile_pool(name="pgi", bufs=1, space="PSUM") as pgi_pool,
            tc.tile_pool(name="pgh", bufs=2, space="PSUM") as pgh_pool,
            tc.tile_pool(name="ptr", bufs=1, space="PSUM") as ptr_pool,
            tc.tile_pool(name="ppb", bufs=2, space="PSUM") as ppb_pool,
            tc.tile_pool(name="dram", bufs=3, space="DRAM") as dram,
            tc.tile_pool(name="dram2", bufs=3, space="DRAM") as dram2,
        ):
            # ---- resident tiles ----
            whh8 = wts.tile([128, HID // 128, GS], fp8)        # 3 KB/part
            wout8 = wts.tile([128, HID // 128, VS], fp8)       # 32 KB/part
            eT = wts.tile([128, EMB // 128, ROWS], bf16)       # 16 KB/part
            wihT = wts.tile([128, EMB // 128, GS], bf16)       # 3 KB/part
            hTst = state.tile([128, SEQ + 1, 4, 2, B], fp8)    # 16.6 KB/part
            gi_rz = state.tile([128, MCH, RZ], bf16)           # 8 KB/part
            gi2n = state.tile([B, SEQ, HSL], bf16)             # 16 KB on 32 parts
            doneT = state.tile([B, SEQ], f32)
            livePB = state.tile([128, MCH], f32)
            padPB = state.tile([128, MCH], f32)
            ident = state.tile([128, 128], bf16)
            identF = state.tile([B, B], f32)
            sacc = state.tile([128, MCH, VT], f32)             # per-vtile exp sums
            sumsF = state.tile([128, MCH], f32)                # per-chunk local sums
            denomT = state.tile([128, MCH], f32)               # global denominators
            scT = state.tile([128, MCH], f32)                  # live/denom scales
            src2s = [state.tile([HSL, AGW], fp8, name=f"src2{i}") for i in range(2)]

            make_identity(nc, ident[:])
            make_identity(nc, identF[:])
            for s2 in src2s:
                nc.vector.memset(s2[:], 0.0)

            # weight loads: big wout8 on the Act HWDGE ring (overlaps with
            # early steps); everything phase-A-critical on the sync ring.
            nc.scalar.dma_start(wout8[:], wout8_d.rearrange("(c p) n -> p c n", p=128))
            nc.sync.dma_start(doneT[:], doneT_d[:])
            nc.sync.dma_start(livePB[:], live_d[:])
            nc.sync.dma_start(padPB[:], pad_d[:])
            nc.sync.dma_start(hTst[:, 0, :, :, :],
                              h8T0_d.rearrange("(j two p) n -> p j two n", two=2, p=128))
            nc.sync.dma_start(whh8[:], whh8_d.rearrange("(c p) n -> p c n", p=128))
            nc.sync.dma_start(wihT[:], wihT_d.rearrange("(c p) n -> p c n", p=128))
            nc.sync.dma_start(eT[:], eT_d.rearrange("(c p) n -> p c n", p=128))

            h_own_init = hown_pool.tile([B, HSL], f32, name="h_own")
            nc.sync.dma_start(h_own_init[:], h0own_d[:])
            h_own = h_own_init

            # ---- phase 0 chunk: gi for steps 4m..4m+3 (values are S*gi) ----
            def gi_chunk(m):
                pgi = pgi_pool.tile([128, GS], f32, name="pgi")
                for k in range(EMB // 128):
                    nc.tensor.matmul(
                        pgi[:], eT[:, k, bass.ts(m, 128)], wihT[:, k, :],
                        start=(k == 0), stop=(k == EMB // 128 - 1),
                    )
                nc.vector.tensor_copy(gi_rz[:, m, :], pgi[:, :RZ])
                tmpn = gtmp.tile([128, HSL], bf16, name="tmpn")
                nc.vector.tensor_copy(tmpn[:], pgi[:, RZ:])
                for i in range(4):
                    t = 4 * m + i
                    p2 = pgi_pool.tile([B, HSL], f32, name="p2")
                    nc.tensor.matmul(p2[:], ident[:, i * B:(i + 1) * B], tmpn[:],
                                     start=True, stop=True)
                    nc.vector.tensor_copy(gi2n[:, t, :], p2[:])

            # ---- phase A single step ----
            def step(t):
                nonlocal h_own
                m, po = t // 4, (t % 4) * B
                bank = pgh_pool.tile([B, GS], f32, name="bank")
                # rz: extract gi rows then accumulate gh (fp8 DoubleRow)
                nc.tensor.matmul(bank[:, :RZ], ident[:, po:po + B], gi_rz[:, m, :],
                                 start=True, stop=False, skip_group_check=True)
                for j in range(4):
                    nc.tensor.matmul(
                        bank[:, :RZ],
                        hTst[:, t, j, :, :],
                        whh8[:, 2 * j:2 * j + 2, :RZ],
                        start=False, stop=(j == 3),
                        perf_mode=DR, skip_group_check=True,
                    )
                # n: gh only (w_hh n-rows are half-scaled on host)
                for j in range(4):
                    nc.tensor.matmul(
                        bank[:, RZ:],
                        hTst[:, t, j, :, :],
                        whh8[:, 2 * j:2 * j + 2, RZ:],
                        start=(j == 0), stop=(j == 3),
                        perf_mode=DR, skip_group_check=True,
                    )
                # trz = tanh(0.5 * (gi+gh));  PSUM holds S*x
                trz = gtmp.tile([B, RZ], f32, name="trz")
                nc.scalar.activation(trz[:], bank[:, :RZ], AF.Tanh, scale=0.5 / S)
                # n = tanh((tr+1)*C + S*gi_n),  C = S*0.5*gh_n
                p1 = gtmp.tile([B, HSL], f32, name="p1")
                nc.vector.tensor_scalar(p1[:], trz[:, :HSL], 1.0, None, ALU.add)
                v = gtmp.tile([B, HSL], f32, name="v")
                nc.vector.tensor_tensor(v[:], p1[:], bank[:, RZ:], ALU.mult)
                nc.vector.tensor_tensor(v[:], v[:], gi2n[:, t, :], ALU.add)
                n_t = gtmp.tile([B, HSL], f32, name="n_t")
                nc.scalar.activation(n_t[:], v[:], AF.Tanh, scale=1.0 / S)
                # z' = max(tanh(z/2), done')  (done' = +-1)
                tzp = gtmp.tile([B, HSL], f32, name="tzp")
                nc.vector.tensor_scalar(tzp[:], trz[:, HSL:], doneT[:, t:t + 1],
                                        None, ALU.max)
                # f = 2*h_new = (n + h) - z'*(n - h)
                s_t = gtmp.tile([B, HSL], f32, name="s_t")
                nc.vector.tensor_tensor(s_t[:], n_t[:], h_own[:], ALU.add)
                d_t = gtmp.tile([B, HSL], f32, name="d_t")
                nc.vector.tensor_tensor(d_t[:], n_t[:], h_own[:], ALU.subtract)
                nc.vector.tensor_tensor(d_t[:], tzp[:], d_t[:], ALU.mult)
                f_t = gtmp.tile([B, HSL], f32, name="f_t")
                nc.vector.tensor_tensor(f_t[:], s_t[:], d_t[:], ALU.subtract)
                h_new = hown_pool.tile([B, HSL], f32, name="h_own")
                nc.vector.tensor_scalar(h_new[:], f_t[:], 0.5, None, ALU.mult)
                h_own = h_new
                # transpose f (f32), cast to fp8 (x 0.5*S_H) in the staging copy
                ptr = ptr_pool.tile([HSL, B], f32, name="ptr")
                nc.tensor.transpose(ptr[:], f_t[:], identF[:])
                src2 = src2s[t % 2]
                nc.vector.tensor_scalar(src2[:, :B], ptr[:], 0.5 * S_H, None,
                                        ALU.mult)
                # piggyback chunk mm's partial softmax sum (raw f32 bytes)
                if t >= 9 and (t - 9) % 4 == 0 and (t - 9) // 4 <= 13:
                    mm = (t - 9) // 4
                    nc.vector.tensor_copy(src2[:, B:].bitcast(f32),
                                          sumsF[:, mm:mm + 1])
                # all-gather
                agin = dram.tile([HSL, AGW], fp8, name="agin")
                agout = dram2.tile([NC * HSL, AGW], fp8, name="agout")
                nc.sync.dma_start(agin[:], src2[:])
                nc.gpsimd.collective_compute(
                    "AllGather", ALU.bypass,
                    replica_groups=[list(range(NC))],
                    ins=[agin.opt()], outs=[agout.opt()],
                )
                ago = agout.rearrange("(j two p) n -> p j two n", two=2, p=128)
                nc.sync.dma_start(hTst[:, t + 1, :, :, :], ago[:, :, :, :B])
                if t >= 9 and (t - 9) % 4 == 0 and (t - 9) // 4 <= 13:
                    mm = (t - 9) // 4
                    s8 = sums8_pool.tile([128, NC, 4], fp8, name="sums8")
                    chunk_sums8[mm] = s8
                    nc.sync.dma_start(
                        s8[:], agout.rearrange("(c p) n -> p c n", p=128)[:, :, B:])

            # ---- phase B vocab tile (chunk m, tile v) ----
            def pb_repack(m):
                hTpb = pbpool.tile([128, 4, 2, 4, B], fp8, name="hTpb")
                chunk_pb[m] = hTpb
                for j in range(4):
                    for two in range(2):
                        nc.vector.tensor_copy(hTpb[:, j, two, :, :],
                                              hTst[:, 4 * m + 1:4 * m + 5, j, two, :])

            def vtile(m, v):
                expb = chunk_expb[m]
                hTpb = chunk_pb[m]
                ppb = ppb_pool.tile([128, VTW], f32, name="ppb")
                for j in range(4):
                    nc.tensor.matmul(
                        ppb[:],
                        hTpb[:, j, :, :, :],
                        wout8[:, 2 * j:2 * j + 2, bass.ts(v, VTW)],
                        start=(j == 0), stop=(j == 3),
                        perf_mode=DR, skip_group_check=True,
                    )
                nc.scalar.activation(expb[:, bass.ts(v, VTW)], ppb[:], AF.Exp,
                                     scale=1.0 / S,
                                     accum_out=sacc[:, m, v:v + 1])

            def chunk_finalize(m):
                nc.vector.reduce_sum(sumsF[:, m:m + 1], sacc[:, m, :], AX.X)

            # ---- normalization, sliced across steps ----
            def norm_phase(m, ph):
                if ph == 0:
                    # chunk m's global sums arrived with gather(4m+9)
                    tmp = gtmp.tile([128, NC], f32, name="dtmp")
                    nc.vector.tensor_copy(tmp[:], chunk_sums8[m][:].bitcast(f32))
                    nc.vector.reduce_sum(denomT[:, m:m + 1], tmp[:], AX.X)
                    inv = gtmp.tile([128, 1], f32, name="inv")
                    nc.vector.reciprocal(inv[:], denomT[:, m:m + 1])
                    nc.vector.tensor_tensor(scT[:, m:m + 1], livePB[:, m:m + 1],
                                            inv[:], ALU.mult)
                    chunk_ouf[m] = outp.tile([128, VS], bf16, name="ouf")
                ouf = chunk_ouf[m]
                sl = bass.ts(ph, NSW)
                nc.vector.tensor_scalar(ouf[:, sl], chunk_expb[m][:, sl],
                                        scT[:, m:m + 1], None, ALU.mult)
                if ph == 0:
                    nc.vector.tensor_tensor(ouf[:, 0:1], ouf[:, 0:1],
                                            padPB[:, m:m + 1], ALU.add)
                if ph == NSL - 1:
                    nc.gpsimd.dma_start(out_d[bass.ts(m, 128), :], ouf[:])

            # ---- main interleaved schedule ----
            chunk_expb = {}
            chunk_ouf = {}
            chunk_pb = {}
            chunk_sums8 = {}
            pending = []
            norm_pending = []
            for t in range(SEQ):
                if t % 4 == 0:
                    gi_chunk(t // 4)
                step(t)
                if t >= 4 and t % 4 == 0:
                    m = (t - 4) // 4
                    chunk_expb[m] = expp.tile([128, VS], bf16, name="expb")
                    pb_repack(m)
                    pending.extend((m, v) for v in range(VT))
                emitted = 0
                while pending and emitted < 3:
                    m, v = pending.pop(0)
                    vtile(m, v)
                    emitted += 1
                    if v == VT - 1:
                        chunk_finalize(m)
                if t >= 10 and (t - 10) % 4 == 0 and (t - 10) // 4 <= 13:
                    norm_pending.extend(((t - 10) // 4, ph) for ph in range(NSL))
                if norm_pending:
                    norm_phase(*norm_pending.pop(0))

            # ---- tail: chunk 15 + one AllReduce for chunks 14, 15 ----
            chunk_expb[15] = expp.tile([128, VS], bf16, name="expb")
            pb_repack(15)
            for m2, v in pending:
                vtile(m2, v)
                if v == VT - 1:
                    chunk_finalize(m2)
            for v in range(VT):
                vtile(15, v)
            chunk_finalize(15)
            arin = dram.tile([128, 2], f32, name="arin")
            arout = dram2.tile([128, 2], f32, name="arout")
            nc.sync.dma_start(arin[:], sumsF[:, 14:16])
            nc.gpsimd.collective_compute(
                "AllReduce", ALU.add,
                replica_groups=[list(range(NC))],
                ins=[arin.opt()], outs=[arout.opt()],
            )
            nc.sync.dma_start(denomT[:, 14:16], arout[:])
            for m, ph in norm_pending:
                norm_phase(m, ph)
            for m in (14, 15):
                inv = gtmp.tile([128, 1], f32, name="inv")
                nc.vector.reciprocal(inv[:], denomT[:, m:m + 1])
                nc.vector.tensor_tensor(scT[:, m:m + 1], livePB[:, m:m + 1],
                                        inv[:], ALU.mult)
                ouf = outp.tile([128, VS], bf16, name="ouf")
                nc.vector.tensor_scalar(ouf[:], chunk_expb[m][:],
                                        scT[:, m:m + 1], None, ALU.mult)
                nc.vector.tensor_tensor(ouf[:, 0:1], ouf[:, 0:1],
                                        padPB[:, m:m + 1], ALU.add)
                nc.gpsimd.dma_start(out_d[bass.ts(m, 128), :], ouf[:])

    nc.compile()
    return nc


def _host_prep(hidden, target, lenseq, emb, w_ih, w_hh, b_ih, b_hh, w_out, b_out):
    assert not np.asarray(b_ih).any() and not np.asarray(b_hh).any() and not np.asarray(b_out).any(), (
        "nonzero biases not supported by this kernel build"
    )
    target = np.asarray(target)
    X = np.empty((SEQ, B), dtype=np.int64)
    X[0] = SOS_ID
    X[1:] = target[:SEQ - 1]
    done = ((X == EOS_ID) | (X == PAD_ID)).astype(np.float32)  # [SEQ, B]
    emb = np.asarray(emb, dtype=np.float32)
    E = emb[X.reshape(-1)]                                     # [2048, 512]
    eT = np.ascontiguousarray(E.T).astype(BF16)                # [512, 2048]
    h0 = np.asarray(hidden, dtype=np.float32)[0]               # [32, 1024]
    h8T0 = np.clip(h0.T * S_H, -240, 240).astype(FP8)   # [HID, B]
    doneT = np.ascontiguousarray((2.0 * done - 1.0).T)         # [B, SEQ], +-1
    done_row = done.reshape(ROWS)
    done_pb = np.ascontiguousarray(done_row.reshape(MCH, 128).T)  # [128, 16]
    live_pb = np.ascontiguousarray(1.0 - done_pb)
    w_ih = np.asarray(w_ih, dtype=np.float32)
    w_hh = np.asarray(w_hh, dtype=np.float32).copy()
    w_out = np.asarray(w_out, dtype=np.float32)
    w_hh[2 * HID:] *= 0.5                                      # fold sigmoid algebra

    in_maps = []
    for c in range(NC):
        rows = np.r_[c * HSL:(c + 1) * HSL,
                     HID + c * HSL:HID + (c + 1) * HSL,
                     2 * HID + c * HSL:2 * HID + (c + 1) * HSL]
        wihT = np.ascontiguousarray(w_ih[rows].T * S).astype(BF16)    # [512, 384]
        whh8 = np.clip(np.ascontiguousarray(w_hh[rows].T) * S_W,
                       -240, 240).astype(FP8)                          # [1024, 384]
        wout8 = np.clip(np.ascontiguousarray(w_out[c * VS:(c + 1) * VS].T) * S_W,
                        -240, 240).astype(FP8)                         # [1024, 4000]
        h0own = np.ascontiguousarray(h0[:, c * HSL:(c + 1) * HSL])
        pad_pb = done_pb if c == 0 else np.zeros_like(done_pb)
        in_maps.append({
            "eT": eT, "wihT": wihT, "whh8": whh8, "wout8": wout8,
            "h8T0": h8T0, "h0own": h0own, "doneT": doneT,
            "livePB": live_pb, "padPB": np.ascontiguousarray(pad_pb),
        })
    return in_maps


def kernel(hidden, target, lenseq, emb, w_ih, w_hh, b_ih, b_hh, w_out, b_out):
    from concourse.bass_utils import run_bass_kernel_spmd

    in_maps = _host_prep(hidden, target, lenseq, emb, w_ih, w_hh, b_ih, b_hh,
                         w_out, b_out)
    if "nc" not in _CACHE:
        _CACHE["nc"] = _build()
    res = run_bass_kernel_spmd(_CACHE["nc"], in_maps, core_ids=list(range(NC)))
    outs = [np.asarray(r["out"], dtype=np.float32) for r in res.results]
    full = np.concatenate(outs, axis=1).reshape(SEQ, B, VOCAB)
    return full[:int(lenseq)]
